# revision 1
# baseline (speedup 1.0000x reference)
# Trainium2 Bass kernel for nn_MeshUnpool (gnn_message_passing).
#
# Reference semantics (per mesh b):
#   idx = cumsum(dst_mask)-1 at true slots; padded[v,:] = mask[v] ? features[:,idx[v]] : 0
#   out = (unroll_mat[b].T @ padded).T / occ  ==  (features[b] @ unroll_mat[b][mask_rows]) / occ
# i.e. the gather+scatter collapses to selecting the E=3072 masked rows of
# unroll_mat, leaving a dense [NF,E] @ [E,U] matmul per mesh, divided
# column-wise by occurrences.  Pure data parallel: one mesh per core.
#
# On-device compute per core:
#   out[128, 4096] = sum_k (A_hi[k] + A_lo[k]).T @ W[k]  * inv_occ
# where A_hi/A_lo is a bf16 hi/lo split of features^T (f32-grade accuracy,
# since bf16*bf16 products are exact in the f32 PSUM accumulator) and W is the
# masked-row-gathered unroll matrix cast to fp8e4m3 (entries are exactly 0/1,
# so the cast is lossless and quarters the dominant HBM traffic; the PE takes
# mixed bf16-stationary x fp8-moving matmuls).  All-zero W rows
# (~6%) are dropped on host, shrinking the contraction further.

import numpy as np
import ml_dtypes

B, NF, E, U = 8, 128, 3072, 4096
NCORES = 8
NT = U // 512          # 8 output column tiles of 512 (one PSUM bank each)

_compiled = {}


def _build_bass(kc):
    """Build + compile the per-core program for a contraction of kc*128 rows."""
    import concourse.bass as bass
    import concourse.bacc as bacc
    import concourse.mybir as mybir
    import concourse.tile as tile

    e = kc * 128
    nc = bacc.Bacc("TRN2", target_bir_lowering=False, debug=False)
    bf16 = mybir.dt.bfloat16
    f32 = mybir.dt.float32

    a_hi = nc.dram_tensor("a_hi", [128, e], bf16, kind="ExternalInput").ap()
    a_lo = nc.dram_tensor("a_lo", [128, e], bf16, kind="ExternalInput").ap()
    fp8 = mybir.dt.float8e4
    w = nc.dram_tensor("w", [e, U], fp8, kind="ExternalInput").ap()
    occ = nc.dram_tensor("occ", [128, U], f32, kind="ExternalInput").ap()
    out = nc.dram_tensor("out", [128, U], f32, kind="ExternalOutput").ap()

    with tile.TileContext(nc) as tc:
        with (
            tc.tile_pool(name="const", bufs=1) as cpool,
            tc.tile_pool(name="wpool", bufs=8) as wpool,
            tc.tile_pool(name="psum", bufs=1, space=bass.MemorySpace.PSUM) as ppool,
            tc.tile_pool(name="opool", bufs=4) as opool,
        ):
            # Everything rides ONE ordered HWDGE ring (sync) so early-phase
            # bytes arrive exactly in consumption order.  Only the first 4
            # chunks of a_hi/a_lo go ahead of w1/w2; the rest defers.
            a_hi_s = cpool.tile([128, e], bf16, tag="ahi")
            a_lo_s = cpool.tile([128, e], bf16, tag="alo")
            occ_s = cpool.tile([128, U], f32, tag="occ")

            # all 8 PSUM banks accumulate in parallel; k-contiguous keeps PE
            # warm.  One tile per bank PAIR (2 banks = [128,1024]) so the
            # epilogue's mul+store covers 2 banks per DVE op (fewer per-op
            # overheads) and depends only on that pair's stop matmuls.
            NP = NT // 2
            psum_pairs = [
                ppool.tile([128, 1024], f32, tag=f"ps{p}", name=f"ps{p}")
                for p in range(NP)
            ]

            def mm(k, n, a_s, start, stop, w_tile):
                nc.tensor.matmul(
                    psum_pairs[n // 2][:, (n % 2) * 512 : (n % 2) * 512 + 512],
                    a_s[:, k * 128 : (k + 1) * 128],
                    w_tile[:, n * 512 : (n + 1) * 512],
                    start=start,
                    stop=stop,
                )

            # host ships A^T chunk-interleaved: a_hi[p, k*128+m] = AT[k*128+p, m]
            # so chunk k's lhsT [K=128, M=128] is a_hi_s[:, k*128:(k+1)*128]
            w_last = None
            ac = min(4, kc)
            for k in range(kc):
                w_t = wpool.tile([128, U], fp8, tag="w")
                if k == 0:
                    # ring order: w0 banks0-3, a_hi/a_lo chunk 0 only, w0
                    # banks 4-7, a chunks 1-3 — the first matmuls wait for
                    # only 256KB + 64KB
                    nc.sync.dma_start(w_t[:, 0:2048], w[0:128, 0:2048])
                    nc.sync.dma_start(a_hi_s[:, 0:128], a_hi[:, 0:128])
                    nc.sync.dma_start(a_lo_s[:, 0:128], a_lo[:, 0:128])
                    nc.sync.dma_start(w_t[:, 2048:U], w[0:128, 2048:U])
                    nc.sync.dma_start(a_hi_s[:, 128 : ac * 128], a_hi[:, 128 : ac * 128])
                    nc.sync.dma_start(a_lo_s[:, 128 : ac * 128], a_lo[:, 128 : ac * 128])
                else:
                    nc.sync.dma_start(w_t[:], w[k * 128 : (k + 1) * 128, :])
                if k == 2:
                    # rest of the stationary operands, behind w1/w2 but well
                    # ahead of their first consumers (chunk 4 matmuls)
                    nc.sync.dma_start(a_hi_s[:, ac * 128 : e], a_hi[:, ac * 128 : e])
                    nc.sync.dma_start(a_lo_s[:, ac * 128 : e], a_lo[:, ac * 128 : e])
                if k == kc // 2:
                    # occ is only needed for the epilogue; it streams
                    # mid-kernel where DMA has slack behind the PE-bound phase
                    nc.sync.dma_start(occ_s[:], occ)
                if k == 0:
                    # banks 0-3 first (piece one), then banks 4-7
                    for lohalf in range(2):
                        for half in range(2):
                            a_s = a_hi_s if half == 0 else a_lo_s
                            for n in range(lohalf * 4, lohalf * 4 + 4):
                                mm(k, n, a_s, start=(half == 0), stop=False, w_tile=w_t)
                elif k < kc - 1:
                    for half in range(2):
                        a_s = a_hi_s if half == 0 else a_lo_s
                        for n in range(NT):
                            mm(k, n, a_s, start=False, stop=False, w_tile=w_t)
                w_last = w_t
            # last chunk: per bank pair do hi,hi,lo,lo then immediately
            # scale+store the pair, overlapping the drain with the remaining
            # pairs' matmuls
            k = kc - 1
            for p in range(NP):
                for half in range(2):
                    a_s = a_hi_s if half == 0 else a_lo_s
                    for n in (2 * p, 2 * p + 1):
                        mm(k, n, a_s, start=False, stop=(half == 1), w_tile=w_last)
                o_t = opool.tile([128, 1024], f32, tag="o")
                nc.vector.tensor_mul(
                    o_t[:], psum_pairs[p][:],
                    occ_s[:, p * 1024 : (p + 1) * 1024],
                )
                nc.sync.dma_start(out[:, p * 1024 : (p + 1) * 1024], o_t[:])

    nc.compile()
    _dedup_ldweights(nc)
    return nc


def _dedup_ldweights(nc):
    """Remove InstLdweights that reload the PE array with the exact weights it
    already holds (consecutive matmuls sharing one stationary operand).  The
    tile legalizer emits one LDWEIGHTS per matmul and neither it nor walrus
    dedups, so 8-matmul groups sharing a lhsT pay 7 redundant ~100ns array
    loads each — pure serial PE time.  Safe here because the stationary tiles
    (bufs=1, written once) are never rewritten mid-kernel.  Any waits/updates
    on a removed LDW are transferred to the next PE instruction."""
    import concourse.mybir as mybir

    for blk in nc.m.functions[0].blocks:
        insts = blk.instructions
        loaded = None
        pending = []  # sync infos of removed LDWs, to merge into next PE inst
        idx = 0
        while idx < len(insts):
            inst = insts[idx]
            if isinstance(inst, mybir.InstLdweights):
                key = (
                    str(inst.ins[0]),
                    str(inst.tile_position),
                    str(inst.perf_mode),
                    str(inst.is_transpose),
                )
                if loaded == key:
                    si = inst.sync_info
                    if si is not None and (si.on_wait or si.on_update):
                        pending.append(si)
                    del insts[idx]
                    continue
                loaded = key
            elif isinstance(inst, mybir.InstMatmult) and pending:
                si = inst.sync_info
                if si is None:
                    si = mybir.SyncInfo(on_wait=[], on_update=[])
                for p in pending:
                    si.on_wait = list(si.on_wait) + list(p.on_wait)
                    si.on_update = list(si.on_update) + list(p.on_update)
                inst.sync_info = si
                pending = []
            idx += 1
        assert not pending, "dangling sync from removed LDWEIGHTS"


def _get_compiled(kc):
    if kc not in _compiled:
        _compiled[kc] = _build_bass(kc)
    return _compiled[kc]


def _prep_cores(features, unroll_mat, occurrences, dst_masks):
    """Host-side prep: mask-gather W rows, drop all-zero rows, hi/lo split of
    features^T, 1/occ broadcast.  Returns (kc, in_maps)."""
    bf16 = ml_dtypes.bfloat16
    per_core = []
    for b in range(B):
        wg = unroll_mat[b][dst_masks[b]]          # [E, U] f32, entries 0/1
        keep = wg.any(axis=1)                      # drop rows with no targets
        wk = wg[keep]
        fk = features[b][:, keep]                  # matching feature columns
        per_core.append((wk, fk))
    kmax = max(w_.shape[0] for w_, _ in per_core)
    kc = (kmax + 127) // 128
    e = kc * 128

    in_maps = []
    for b in range(B):
        wk, fk = per_core[b]
        r = wk.shape[0]
        wpad = np.zeros((e, U), dtype=ml_dtypes.float8_e4m3)
        wpad[:r] = wk.astype(ml_dtypes.float8_e4m3)  # 0/1 -> exact even in fp8
        at = np.zeros((e, 128), dtype=np.float32)  # A^T, zero-padded rows
        at[:r] = fk.T
        hi = at.astype(bf16)
        lo = (at - hi.astype(np.float32)).astype(bf16)

        def interleave(x):  # [e,128] -> [128,e]; col k*128+m holds x[k*128+p, m]
            return np.ascontiguousarray(
                x.reshape(kc, 128, 128).transpose(1, 0, 2).reshape(128, e)
            )

        inv_occ = (1.0 / occurrences[b].reshape(U).astype(np.float32)).astype(
            np.float32
        )
        in_maps.append(
            {
                "a_hi": interleave(hi),
                "a_lo": interleave(lo),
                "w": wpad,
                "occ": np.ascontiguousarray(np.broadcast_to(inv_occ, (128, U))),
            }
        )
    return kc, in_maps


def kernel(features, unroll_mat, occurrences, dst_masks):
    import concourse.bass_utils as bass_utils

    features = np.asarray(features, dtype=np.float32)
    unroll_mat = np.asarray(unroll_mat, dtype=np.float32)
    occurrences = np.asarray(occurrences, dtype=np.float32)
    dst_masks = np.asarray(dst_masks).astype(bool)

    kc, in_maps = _prep_cores(features, unroll_mat, occurrences, dst_masks)
    nc = _get_compiled(kc)
    try:
        res = bass_utils.run_bass_kernel_spmd(
            nc, in_maps, core_ids=list(range(NCORES))
        )
    except Exception:
        # one retry for transient device hiccups (e.g. a wedged exec unit)
        res = bass_utils.run_bass_kernel_spmd(
            nc, in_maps, core_ids=list(range(NCORES))
        )
    return np.stack([res.results[b]["out"] for b in range(B)], axis=0)



# revision 2
# speedup vs baseline: 1.4730x; 1.4730x over previous
# Trainium2 Bass kernel for nn_MeshUnpool (gnn_message_passing).
#
# Reference semantics (per mesh b):
#   idx = cumsum(dst_mask)-1 at true slots; padded[v,:] = mask[v] ? features[:,idx[v]] : 0
#   out = (unroll_mat[b].T @ padded).T / occ  ==  (features[b] @ unroll_mat[b][mask_rows]) / occ
# i.e. the gather+scatter collapses to selecting the E=3072 masked rows of
# unroll_mat, leaving a dense [NF,E] @ [E,U] matmul per mesh, divided
# column-wise by occurrences.  Pure data parallel: one mesh per core.
#
# On-device compute per core (v2, fp8 DoubleRow):
#   out[128, ncol] = sum_k (A_hi[k] + A_lo[k]).T @ W[k]
# with BOTH operands fp8e4m3 so the PE runs MatmulPerfMode.DoubleRow: each
# instruction contracts TWO 128-row chunks at 0.5 cycles per output column
# (vs 1.0 for the old bf16-stationary path) -- the PE stops being the
# bottleneck.  A is a fp8 hi/lo split of features^T (hi + lo recovers ~7
# mantissa bits; validated 1.9e-3 global rel err vs the 2e-2 budget).  W is
# the masked-row gather of unroll_mat cast to fp8 (entries 0/1, lossless).
#
# HBM traffic cuts vs v1 (this is the memory-bound regime):
#   - all-zero ROWS of W dropped on host (~6%)          [v1 already had this]
#   - all-zero COLUMNS of W dropped on host (~11%); host scatters the
#     computed columns back into the full [128, 4096] output
#   - occurrences never shipped: the division happens on host post-gather
#   - A ships as fp8 hi/lo (was bf16 hi/lo): 0.75 MB instead of 1.5 MB
#   - out ships as bf16 (harness tolerance is 2e-2): 0.93 MB instead of 2 MB
#   => ~12.4 MB/core instead of ~17.8 MB/core
#   - W stream alternates between the two HWDGE queues (SP + Activation) so
#     both DMA pipelines stay full.

import numpy as np
import ml_dtypes

B, NF, E, U = 8, 128, 3072, 4096
NCORES = 8

_compiled = {}


def _build_bass(kc, ncol):
    """Per-core program: contraction of kc*128 (zero-padded) rows over ncol
    kept output columns.  kc may be odd: the last chunk is processed FIRST
    with plain fp8 matmuls, then chunk pairs run in DoubleRow mode."""
    import concourse.bass as bass
    import concourse.bacc as bacc
    import concourse.mybir as mybir
    import concourse.tile as tile

    nc = bacc.Bacc("TRN2", target_bir_lowering=False, debug=False)
    fp8 = mybir.dt.float8e4
    f32 = mybir.dt.float32
    bf16 = mybir.dt.bfloat16
    DR = mybir.MatmulPerfMode.DoubleRow

    # Host ships A^T chunk-interleaved ([128, kc, 128]: a[p, k, m] =
    # AT[k*128+p, m]) and W partition-major ([128, kc, ncol]: w[p, k, :] =
    # W[k*128+p, :]) so chunk PAIRS are contiguous per partition and slice
    # directly into the [part, 2, free] APs DoubleRow wants.
    a_hi = nc.dram_tensor("a_hi", [128, kc, 128], fp8, kind="ExternalInput").ap()
    a_lo = nc.dram_tensor("a_lo", [128, kc, 128], fp8, kind="ExternalInput").ap()
    w = nc.dram_tensor("w", [128, kc, ncol], fp8, kind="ExternalInput").ap()
    out = nc.dram_tensor("out", [128, ncol], bf16, kind="ExternalOutput").ap()

    npairs = kc // 2
    tail = kc % 2
    # block sequence: optional odd tail chunk first, then DoubleRow pairs
    blocks = ([("tail", kc - 1)] if tail else []) + [("pair", j) for j in range(npairs)]
    nb = len(blocks)

    # PSUM column tiles of up to 1024 (2 banks); matmul slices of up to 512
    # (1 bank) never cross a bank boundary.
    ptiles = []
    off = 0
    while off < ncol:
        wd = min(1024, ncol - off)
        ptiles.append((off, wd))
        off += wd
    slices = []
    off = 0
    while off < ncol:
        wd = min(512, ncol - off)
        slices.append((off, wd))
        off += wd

    def locate(coff):
        for i, (o, wd) in enumerate(ptiles):
            if o <= coff < o + wd:
                return i, coff - o
        raise AssertionError(coff)

    with tile.TileContext(nc) as tc:
        with (
            tc.tile_pool(name="apool", bufs=1) as apool,
            tc.tile_pool(name="wpool", bufs=6) as wpool,
            tc.tile_pool(name="psum", bufs=1, space=bass.MemorySpace.PSUM) as ppool,
            tc.tile_pool(name="opool", bufs=4) as opool,
        ):
            a_hi_s = apool.tile([128, kc, 128], fp8, tag="ahi")
            a_lo_s = apool.tile([128, kc, 128], fp8, tag="alo")
            psums = [
                ppool.tile([128, wd], f32, tag=f"ps{i}", name=f"ps{i}")
                for i, (o, wd) in enumerate(ptiles)
            ]

            def mm(a_s, ks, nk, pm, w_t, coff, cw, start, stop):
                ti, lo = locate(coff)
                nc.tensor.matmul(
                    psums[ti][:, lo : lo + cw],
                    a_s[:, ks : ks + nk, :],
                    w_t[:, 0:nk, coff : coff + cw],
                    start=start,
                    stop=stop,
                    perf_mode=pm,
                )

            for bi, (kind, idx) in enumerate(blocks):
                if kind == "tail":
                    ks, nk, pm = idx, 1, None
                else:
                    ks, nk, pm = 2 * idx, 2, DR
                w_t = wpool.tile([128, 2, ncol], fp8, tag="w")
                if bi == 0:
                    # prologue: first W block split by columns so the first
                    # matmuls wait on ~256KB, A staged on the Act queue in
                    # consumption order (first block's chunks, then the rest)
                    c0 = 2048 // nk
                    nc.sync.dma_start(w_t[:, 0:nk, 0:c0], w[:, ks : ks + nk, 0:c0])
                    nc.scalar.dma_start(
                        a_hi_s[:, ks : ks + nk, :], a_hi[:, ks : ks + nk, :]
                    )
                    nc.scalar.dma_start(
                        a_lo_s[:, ks : ks + nk, :], a_lo[:, ks : ks + nk, :]
                    )
                    nc.sync.dma_start(w_t[:, 0:nk, c0:ncol], w[:, ks : ks + nk, c0:ncol])
                    if tail:
                        nc.scalar.dma_start(a_hi_s[:, 0:idx, :], a_hi[:, 0:idx, :])
                        nc.scalar.dma_start(a_lo_s[:, 0:idx, :], a_lo[:, 0:idx, :])
                    else:
                        nc.scalar.dma_start(a_hi_s[:, 2:kc, :], a_hi[:, 2:kc, :])
                        nc.scalar.dma_start(a_lo_s[:, 2:kc, :], a_lo[:, 2:kc, :])
                else:
                    qe = nc.sync if bi % 2 == 0 else nc.scalar
                    qe.dma_start(w_t[:, 0:nk, :], w[:, ks : ks + nk, :])

                if bi < nb - 1:
                    for half in range(2):
                        a_s = a_hi_s if half == 0 else a_lo_s
                        for coff, cw in slices:
                            mm(
                                a_s, ks, nk, pm, w_t, coff, cw,
                                start=(bi == 0 and half == 0),
                                stop=False,
                            )
                else:
                    # final block: finish per PSUM tile, evict to bf16 and
                    # store while the remaining tiles' matmuls drain
                    for t, (toff, twd) in enumerate(ptiles):
                        tsl = [(c, cw) for c, cw in slices if toff <= c < toff + twd]
                        for half in range(2):
                            a_s = a_hi_s if half == 0 else a_lo_s
                            for coff, cw in tsl:
                                mm(
                                    a_s, ks, nk, pm, w_t, coff, cw,
                                    start=(bi == 0 and half == 0),
                                    stop=(half == 1),
                                )
                        o_t = opool.tile([128, 1024], bf16, tag="o")
                        nc.vector.tensor_copy(o_t[:, 0:twd], psums[t][:])
                        q2 = nc.sync if t % 2 == 0 else nc.scalar
                        q2.dma_start(out[:, toff : toff + twd], o_t[:, 0:twd])

    nc.compile()
    _dedup_ldweights(nc)
    return nc


def _dedup_ldweights(nc):
    """Remove InstLdweights that reload the PE array with the exact weights it
    already holds (consecutive matmuls sharing one stationary operand).  The
    tile legalizer emits one LDWEIGHTS per matmul and neither it nor walrus
    dedups, so slice groups sharing a lhsT pay redundant ~100ns array loads
    each -- pure serial PE time.  Safe here because the stationary tiles
    (bufs=1, written once) are never rewritten mid-kernel.  Any waits/updates
    on a removed LDW are transferred to the next PE instruction."""
    import concourse.mybir as mybir

    for blk in nc.m.functions[0].blocks:
        insts = blk.instructions
        loaded = None
        pending = []  # sync infos of removed LDWs, to merge into next PE inst
        idx = 0
        while idx < len(insts):
            inst = insts[idx]
            if isinstance(inst, mybir.InstLdweights):
                key = (
                    str(inst.ins[0]),
                    str(inst.tile_position),
                    str(inst.perf_mode),
                    str(inst.is_transpose),
                )
                if loaded == key:
                    si = inst.sync_info
                    if si is not None and (si.on_wait or si.on_update):
                        pending.append(si)
                    del insts[idx]
                    continue
                loaded = key
            elif isinstance(inst, mybir.InstMatmult) and pending:
                si = inst.sync_info
                if si is None:
                    si = mybir.SyncInfo(on_wait=[], on_update=[])
                for p in pending:
                    si.on_wait = list(si.on_wait) + list(p.on_wait)
                    si.on_update = list(si.on_update) + list(p.on_update)
                inst.sync_info = si
                pending = []
            idx += 1
        assert not pending, "dangling sync from removed LDWEIGHTS"


def _get_compiled(kc, ncol):
    if (kc, ncol) not in _compiled:
        _compiled[(kc, ncol)] = _build_bass(kc, ncol)
    return _compiled[(kc, ncol)]


def _prep_cores(features, unroll_mat, occurrences, dst_masks):
    """Host-side prep: mask-gather W rows, drop all-zero rows AND columns,
    fp8 hi/lo split of features^T.  Returns (kc, ncol, in_maps, meta)."""
    f8 = ml_dtypes.float8_e4m3
    per = []
    for b in range(B):
        wg = unroll_mat[b][dst_masks[b]]          # [E, U] f32, entries 0/1
        keep = wg.any(axis=1)                      # drop rows with no targets
        wk = wg[keep]
        fk = features[b][:, keep]                  # matching feature columns
        colidx = np.where(wk.any(axis=0))[0]       # drop all-zero output cols
        per.append((wk[:, colidx], fk, colidx))
    rmax = max(w_.shape[0] for w_, _, _ in per)
    cmax = max(w_.shape[1] for w_, _, _ in per)
    kc = (rmax + 127) // 128
    e = kc * 128
    ncol = ((cmax + 31) // 32) * 32

    in_maps, meta = [], []
    for b in range(B):
        wkc, fk, colidx = per[b]
        r, c = wkc.shape
        wpad = np.zeros((e, ncol), dtype=f8)
        wpad[:r, :c] = wkc.astype(f8)              # 0/1 -> exact even in fp8
        w3 = np.ascontiguousarray(wpad.reshape(kc, 128, ncol).transpose(1, 0, 2))
        at = np.zeros((e, 128), dtype=np.float32)  # A^T, zero-padded rows
        at[:r] = fk.T
        hi = at.astype(f8)
        lo = (at - hi.astype(np.float32)).astype(f8)

        def inter(x):  # [e,128] -> [128,kc,128]; a[p,k,m] = x[k*128+p, m]
            return np.ascontiguousarray(x.reshape(kc, 128, 128).transpose(1, 0, 2))

        in_maps.append({"a_hi": inter(hi), "a_lo": inter(lo), "w": w3})
        meta.append((colidx, c))
    return kc, ncol, in_maps, meta


def kernel(features, unroll_mat, occurrences, dst_masks):
    import concourse.bass_utils as bass_utils

    features = np.asarray(features, dtype=np.float32)
    unroll_mat = np.asarray(unroll_mat, dtype=np.float32)
    occurrences = np.asarray(occurrences, dtype=np.float32)
    dst_masks = np.asarray(dst_masks).astype(bool)

    kc, ncol, in_maps, meta = _prep_cores(features, unroll_mat, occurrences, dst_masks)
    nc = _get_compiled(kc, ncol)
    try:
        res = bass_utils.run_bass_kernel_spmd(
            nc, in_maps, core_ids=list(range(NCORES))
        )
    except Exception:
        # one retry for transient device hiccups (e.g. a wedged exec unit)
        res = bass_utils.run_bass_kernel_spmd(
            nc, in_maps, core_ids=list(range(NCORES))
        )
    occ = occurrences.reshape(B, U)
    full = np.zeros((B, NF, U), dtype=np.float32)
    for b in range(B):
        colidx, c = meta[b]
        dev = np.asarray(res.results[b]["out"])[:, :c].astype(np.float32)
        full[b][:, colidx] = dev / occ[b, colidx][None, :]
    return full


# revision 7
# speedup vs baseline: 1.7406x; 1.1817x over previous
# Trainium2 Bass kernel for nn_MeshUnpool (gnn_message_passing).
#
# Reference semantics (per mesh b):
#   idx = cumsum(dst_mask)-1 at true slots; padded[v,:] = mask[v] ? features[:,idx[v]] : 0
#   out = (unroll_mat[b].T @ padded).T / occ  ==  (features[b] @ unroll_mat[b][mask_rows]) / occ
# i.e. the gather+scatter collapses to selecting the E=3072 masked rows of
# unroll_mat, leaving a dense [NF,E] @ [E,U] matmul per mesh, divided
# column-wise by occurrences.  Pure data parallel: one mesh per core.
#
# On-device compute per core (v3):
#   out[128, ncol] = sum_k A[k].T @ W[k]
# A = features^T in fp16 (stationary; 0.05% rounding), W = the masked-row
# gather of unroll_mat cast to fp8 (entries 0/1, lossless; moving operand at
# 1 byte/elem).  The PE runs 1 cycle per output column per 128-row chunk for
# any <=16-bit dtype pair, so a single fp16 pass costs the same cycles as any
# hi/lo split scheme -- and half of what the old bf16 hi/lo baseline paid.
#
# HBM traffic (memory-bound regime): ~12.4 MB/core
#   - all-zero ROWS of W dropped on host (~6%)
#   - all-zero COLUMNS of W dropped on host (~11%); host scatters computed
#     columns back into the full [128, 4096] output
#   - occurrences never shipped: division happens on host after the gather
#   - A ships fp16 (0.75 MB), out ships fp16 (0.92 MB)
#   - W stream alternates between the two HWDGE queues (SP + Activation);
#     A rides the Act queue in small just-in-time pieces so it never blocks
#     the next W chunk (v2 lost 8us to exactly that).

import numpy as np
import ml_dtypes

B, NF, E, U = 8, 128, 3072, 4096
NCORES = 8

_compiled = {}


def _build_bass(kc, ncol):
    """Per-core program: kc 128-row chunks contracted over ncol kept output
    columns, one fp16(stationary) x fp8(moving) matmul pass per chunk."""
    import concourse.bass as bass
    import concourse.bacc as bacc
    import concourse.mybir as mybir
    import concourse.tile as tile

    nc = bacc.Bacc("TRN2", target_bir_lowering=False, debug=False)
    fp8 = mybir.dt.float8e4
    f16 = mybir.dt.float16
    f32 = mybir.dt.float32

    # Host ships A^T chunk-interleaved ([128, kc, 128]: a[p, k, m] =
    # AT[k*128+p, m]) and W partition-major ([128, kc, ncol]: w[p, k, :] =
    # W[k*128+p, :]) so chunk k slices directly into [part, free] APs with
    # per-partition-contiguous DMA rows.
    a = nc.dram_tensor("a", [128, kc, 128], f16, kind="ExternalInput").ap()
    w = nc.dram_tensor("w", [128, kc, ncol], fp8, kind="ExternalInput").ap()
    out = nc.dram_tensor("out", [128, ncol], f16, kind="ExternalOutput").ap()

    # PSUM column tiles of up to 1024 (2 banks); matmul slices of up to 512
    # (1 bank) never cross a bank boundary.
    ptiles = []
    off = 0
    while off < ncol:
        wd = min(1024, ncol - off)
        ptiles.append((off, wd))
        off += wd
    slices = []
    off = 0
    while off < ncol:
        wd = min(512, ncol - off)
        slices.append((off, wd))
        off += wd

    def locate(coff):
        for i, (o, wd) in enumerate(ptiles):
            if o <= coff < o + wd:
                return i, coff - o
        raise AssertionError(coff)

    # A arrives in 4-chunk pieces on the Act queue, interleaved between the
    # odd W chunks so each piece lands well before its chunks' matmuls but
    # never delays the W stream by more than ~0.5us.  Chunk 0's stationary
    # leads the Sync queue instead (so w1 can lead Act and close the early
    # PE gap waiting on chunk 1).
    apieces = [(1, min(5, kc))]
    while apieces[-1][1] < kc:
        s = apieces[-1][1]
        apieces.append((s, min(s + 4, kc)))

    with tile.TileContext(nc) as tc:
        with (
            tc.tile_pool(name="apool", bufs=1) as apool,
            tc.tile_pool(name="wpool", bufs=6) as wpool,
            tc.tile_pool(name="psum", bufs=1, space=bass.MemorySpace.PSUM) as ppool,
            tc.tile_pool(name="opool", bufs=4) as opool,
        ):
            a_s = apool.tile([128, kc, 128], f16, tag="a")
            z_t = apool.tile([128, 512], fp8, tag="z")
            psums = [
                ppool.tile([128, wd], f32, tag=f"ps{i}", name=f"ps{i}")
                for i, (o, wd) in enumerate(ptiles)
            ]

            # PE p-state warmup: ~3us of throwaway matmuls on a zeroed tile
            # run while the W prologue streams, so the first real matmuls
            # start at the full 2.4 GHz clock instead of the cold 0.65 GHz.
            nc.vector.memset(z_t[:], 0)
            for _ in range(8):
                nc.tensor.matmul(
                    psums[0][:, 0:512], z_t[:, 0:128], z_t[:], start=True, stop=True
                )

            def mm(k, w_t, coff, cw, start, stop):
                ti, lo = locate(coff)
                nc.tensor.matmul(
                    psums[ti][:, lo : lo + cw],
                    a_s[:, k : k + 1, :],
                    w_t[:, coff : coff + cw],
                    start=start,
                    stop=stop,
                )

            na = 0  # next A piece to enqueue on the act queue
            for k in range(kc):
                w_t = wpool.tile([128, ncol], fp8, tag="w")
                if k == 0:
                    # prologue: chunk 0's stationary + first W chunk (split by
                    # columns) lead the Sync queue; w1 leads Act concurrently
                    nc.sync.dma_start(a_s[:, 0:1, :], a[:, 0:1, :])
                    nc.sync.dma_start(w_t[:, 0:1024], w[:, 0, 0:1024])
                    nc.sync.dma_start(w_t[:, 1024:ncol], w[:, 0, 1024:ncol])
                elif k % 2 == 0:
                    nc.sync.dma_start(w_t[:], w[:, k, :])
                else:
                    nc.scalar.dma_start(w_t[:], w[:, k, :])
                    if na < len(apieces):
                        s, e_ = apieces[na]
                        nc.scalar.dma_start(a_s[:, s:e_, :], a[:, s:e_, :])
                        na += 1

                if k < kc - 1:
                    for coff, cw in slices:
                        mm(k, w_t, coff, cw, start=(k == 0), stop=False)
                else:
                    # final chunk: finish per PSUM tile, evict to fp16 and
                    # store while the remaining tiles' matmuls drain; casts
                    # alternate DVE/Act so the two evict chains run in parallel
                    for t, (toff, twd) in enumerate(ptiles):
                        for coff, cw in slices:
                            if toff <= coff < toff + twd:
                                mm(k, w_t, coff, cw, start=(k == 0), stop=True)
                        o_t = opool.tile([128, 1024], f16, tag="o")
                        if t % 2 == 0:
                            nc.vector.tensor_copy(o_t[:, 0:twd], psums[t][:])
                        else:
                            nc.scalar.copy(o_t[:, 0:twd], psums[t][:])
                        q2 = nc.sync if t % 2 == 0 else nc.scalar
                        q2.dma_start(out[:, toff : toff + twd], o_t[:, 0:twd])

    nc.compile()
    _dedup_ldweights(nc)
    return nc


def _dedup_ldweights(nc):
    """Remove InstLdweights that reload the PE array with the exact weights it
    already holds (consecutive matmuls sharing one stationary operand).  The
    tile legalizer emits one LDWEIGHTS per matmul and neither it nor walrus
    dedups, so slice groups sharing a lhsT pay redundant ~100ns array loads
    each -- pure serial PE time.  Safe here because the stationary tiles
    (bufs=1, written once) are never rewritten mid-kernel.  Any waits/updates
    on a removed LDW are transferred to the next PE instruction."""
    import concourse.mybir as mybir

    for blk in nc.m.functions[0].blocks:
        insts = blk.instructions
        loaded = None
        pending = []  # sync infos of removed LDWs, to merge into next PE inst
        idx = 0
        while idx < len(insts):
            inst = insts[idx]
            if isinstance(inst, mybir.InstLdweights):
                key = (
                    str(inst.ins[0]),
                    str(inst.tile_position),
                    str(inst.perf_mode),
                    str(inst.is_transpose),
                )
                if loaded == key:
                    si = inst.sync_info
                    if si is not None and (si.on_wait or si.on_update):
                        pending.append(si)
                    del insts[idx]
                    continue
                loaded = key
            elif isinstance(inst, mybir.InstMatmult) and pending:
                si = inst.sync_info
                if si is None:
                    si = mybir.SyncInfo(on_wait=[], on_update=[])
                for p in pending:
                    si.on_wait = list(si.on_wait) + list(p.on_wait)
                    si.on_update = list(si.on_update) + list(p.on_update)
                inst.sync_info = si
                pending = []
            idx += 1
        assert not pending, "dangling sync from removed LDWEIGHTS"


def _get_compiled(kc, ncol):
    if (kc, ncol) not in _compiled:
        _compiled[(kc, ncol)] = _build_bass(kc, ncol)
    return _compiled[(kc, ncol)]


def _prep_cores(features, unroll_mat, occurrences, dst_masks):
    """Host-side prep: mask-gather W rows, drop all-zero rows AND columns,
    fp16 cast of features^T.  Returns (kc, ncol, in_maps, meta)."""
    f8 = ml_dtypes.float8_e4m3
    per = []
    for b in range(B):
        wg = unroll_mat[b][dst_masks[b]]          # [E, U] f32, entries 0/1
        keep = wg.any(axis=1)                      # drop rows with no targets
        wk = wg[keep]
        fk = features[b][:, keep]                  # matching feature columns
        colidx = np.where(wk.any(axis=0))[0]       # drop all-zero output cols
        per.append((wk[:, colidx], fk, colidx))
    rmax = max(w_.shape[0] for w_, _, _ in per)
    cmax = max(w_.shape[1] for w_, _, _ in per)
    kc = (rmax + 127) // 128
    e = kc * 128
    ncol = ((cmax + 31) // 32) * 32

    in_maps, meta = [], []
    for b in range(B):
        wkc, fk, colidx = per[b]
        r, c = wkc.shape
        wpad = np.zeros((e, ncol), dtype=f8)
        wpad[:r, :c] = wkc.astype(f8)              # 0/1 -> exact even in fp8
        w3 = np.ascontiguousarray(wpad.reshape(kc, 128, ncol).transpose(1, 0, 2))
        at = np.zeros((e, 128), dtype=np.float32)  # A^T, zero-padded rows
        at[:r] = fk.T
        a3 = np.ascontiguousarray(
            at.astype(np.float16).reshape(kc, 128, 128).transpose(1, 0, 2)
        )
        in_maps.append({"a": a3, "w": w3})
        meta.append((colidx, c))
    return kc, ncol, in_maps, meta


def kernel(features, unroll_mat, occurrences, dst_masks):
    import concourse.bass_utils as bass_utils

    features = np.asarray(features, dtype=np.float32)
    unroll_mat = np.asarray(unroll_mat, dtype=np.float32)
    occurrences = np.asarray(occurrences, dtype=np.float32)
    dst_masks = np.asarray(dst_masks).astype(bool)

    kc, ncol, in_maps, meta = _prep_cores(features, unroll_mat, occurrences, dst_masks)
    nc = _get_compiled(kc, ncol)
    try:
        res = bass_utils.run_bass_kernel_spmd(
            nc, in_maps, core_ids=list(range(NCORES))
        )
    except Exception:
        # one retry for transient device hiccups (e.g. a wedged exec unit)
        res = bass_utils.run_bass_kernel_spmd(
            nc, in_maps, core_ids=list(range(NCORES))
        )
    occ = occurrences.reshape(B, U)
    full = np.zeros((B, NF, U), dtype=np.float32)
    for b in range(B):
        colidx, c = meta[b]
        dev = np.asarray(res.results[b]["out"])[:, :c].astype(np.float32)
        full[b][:, colidx] = dev / occ[b, colidx][None, :]
    return full
